# revision 9
# baseline (speedup 1.0000x reference)
"""Trainium2 Bass kernel for nn_CustomLoss_38062000177852.

Computes: CE(logits, tgt) + overlap_penalty(argmax(logits), sizes) for
logits [32,1024,1024] f32, tgt [32,1024] i32, sizes [32,1024] i32.

Sharding: batch dim (32) split 4-per-core across 8 NeuronCores (SPMD, one
Bass program, per-core input shards). Each core returns two partial sums
(ce_sum, overlap_count); host combines: loss = -ce/(B*T) + count/B.

Per-core layout: 4096 rows (b,t) -> 32 blocks of 128 rows. Row (b,t) lives
at partition p = t%128 of block k = b*8 + t//128 (flat row k*128+p).

v2 engine schedule (per streamed block, ~1.4 us steady state):
  DMA  : xk [128,1024] f32 HBM->SBUF (ring of 4).
  ACT  : exps16 = Exp(xk) in fp16 with f32 accum -> sum-exp. fp16 exp is
         monotone in x, so all later max/argmax run on exps16 at DVE 2x rate.
  DVE  : max8(exps16) -> top-8, max_index -> argmax column (2x fp16).
  DVE/Pool (split): x[tgt] extraction as a fused mask-dot:
         XGE[p,k] = sum_v [v==tgt] * exps16[p,v]  (= exp(x[tgt]));
         x[tgt] = Ln(XGE) later on ACT. One scalar_tensor_tensor pass.
  Pool : sizes[b, perm] softDGE indirect gather (128 offsets/call).
CE = sum(Ln(XGE) - Ln(sum-exp)); then the offset scan + pair count (below).

Offset recurrence (reference scan): e_t = s_t + same_t*max(e_{t-1}-700, 0)
rewritten as e_t = max(e_{t-1} + a_t, b_t), a_t = same_t ? s_t-700 : -BIG,
b_t = s_t  -- a (max,+) linear scan, computed hierarchically: per-chunk scan
([32,128], t on free dim), chunk-map composition scan over 32 chunks, then
re-scan with per-chunk initial states. Exact in f32 (all values < 2^24).

Overlap count: pairs (t, t-d) need 700d < e_{t-d} - offs_t, so only d <= W
can overlap (adjacent d=1 provably never overlaps). Counted with shifted-AP
compares for d in [2, W].
"""
import numpy as np

import concourse.bacc as bacc
import concourse.bass as bass
import concourse.mybir as mybir
import concourse.tile as tile
from concourse import bass_utils
from concourse.masks import make_identity

f32 = mybir.dt.float32
f16 = mybir.dt.float16
i32 = mybir.dt.int32
u32 = mybir.dt.uint32
ALU = mybir.AluOpType
AX = mybir.AxisListType
ACTF = mybir.ActivationFunctionType

B, T, V = 32, 1024, 1024
NCORES = 8
BC = B // NCORES              # batches per core
NBLK = BC * (T // 128)        # 32 row-blocks per core
P = 128
TAKT = 700.0
BIG = 1.0e6                   # absorbing "minus infinity" for the scan input
NEG = -1.0e30                 # scan initial state
W = 6                         # max pair distance checked (d in [2, W])
XG_POOL = 0                   # blocks whose x[tgt] mask-dot runs on Pool
                              # (codegen rejects TensorScalarPtr on Pool)


def _build_program():
    nc = bacc.Bacc("TRN2", debug=False)

    lg = nc.dram_tensor("logits", [BC, T, V], f32, kind="ExternalInput")
    tg = nc.dram_tensor("tgt", [BC, T], i32, kind="ExternalInput")
    sz = nc.dram_tensor("sizes", [BC, V], i32, kind="ExternalInput")
    outd = nc.dram_tensor("out", [1, 2], f32, kind="ExternalOutput")

    lgf = lg.ap().rearrange("b t v -> (b t) v")          # [4096, 1024]
    lgflat = lg.ap().rearrange("b t v -> (b t v)").rearrange("(n o) -> n o", o=1)
    szflat = sz.ap().rearrange("b v -> (b v)").rearrange("(n o) -> n o", o=1)

    with tile.TileContext(nc) as tc:
        with (
            tc.tile_pool(name="sb", bufs=1) as sb,
            tc.tile_pool(name="xring", bufs=6) as xring,
            tc.tile_pool(name="ering", bufs=4) as ering,
            tc.tile_pool(name="ps", bufs=1, space="PSUM") as ps,
        ):
            # ---------------- constants / early independent work ----------
            ident = sb.tile([P, P], f32)
            make_identity(nc, ident)
            ones128 = sb.tile([P, 1], f32)
            nc.vector.memset(ones128[:], 1.0)
            ones11 = sb.tile([1, 1], f32)
            nc.vector.memset(ones11[:], 1.0)

            # tgt in [p, (b,c)] layout via strided DMA
            TGT = sb.tile([P, NBLK], i32)
            nc.sync.dma_start(
                out=TGT[:].rearrange("p (b c) -> p b c", b=BC),
                in_=tg.ap().rearrange("b (c p) -> p b c", p=P),
            )
            # x[tgt] gather offsets: flat = (k*128+p)*1024 + tgt
            OFB = sb.tile([P, NBLK], i32)
            nc.gpsimd.iota(OFB[:], pattern=[[P, NBLK]], base=0,
                           channel_multiplier=1)
            nc.vector.tensor_scalar(out=OFB[:], in0=OFB[:], scalar1=float(V),
                                    scalar2=None, op0=ALU.mult)
            OFFX = sb.tile([P, NBLK], i32)
            nc.vector.tensor_tensor(out=OFFX[:], in0=OFB[:], in1=TGT[:], op=ALU.add)
            XG = sb.tile([P, NBLK], f32)
            # front-load all 32 x[tgt] gathers on Pool (independent of compute,
            # so they fill Pool's idle head while SZG gathers trickle in later)
            for k in range(NBLK):
                nc.gpsimd.indirect_dma_start(
                    out=XG[:, k:k + 1], out_offset=None, in_=lgflat,
                    in_offset=bass.IndirectOffsetOnAxis(ap=OFFX[:, k:k + 1], axis=0),
                )

            # b*1024 iota (batch id base for sizes gather / perm augmentation)
            BIOT = sb.tile([P, NBLK], i32)
            nc.gpsimd.iota(BIOT[:].rearrange("p (b c) -> p b c", b=BC),
                           pattern=[[T, BC], [0, NBLK // BC]], base=0,
                           channel_multiplier=0)

            # u*700 grid in [32, 128] layout (u = k*128 + f)
            UI = sb.tile([NBLK, P], i32)
            nc.gpsimd.iota(UI[:], pattern=[[1, P]], base=0, channel_multiplier=P)
            U700 = sb.tile([NBLK, P], f32)
            nc.vector.tensor_scalar(out=U700[:], in0=UI[:], scalar1=TAKT,
                                    scalar2=None, op0=ALU.mult)

            # ---------------- phase 1: stream logits -----------------------
            EM8 = sb.tile([P, NBLK, 8], f16)
            IDX8 = sb.tile([P, NBLK, 8], u32)
            SUME = sb.tile([P, NBLK], f32)
            SIDX = sb.tile([P, NBLK], i32)
            SZG = sb.tile([P, NBLK], i32)

            for k in range(NBLK):
                xk = xring.tile([P, V], f32, tag="x")
                nc.sync.dma_start(out=xk[:], in_=lgf[k * P:(k + 1) * P, :])
                e16 = ering.tile([P, V], f16, tag="e")
                nc.scalar.activation(out=e16[:], in_=xk[:], func=ACTF.Exp,
                                     bias=0.0, scale=1.0,
                                     accum_out=SUME[:, k:k + 1])
                nc.vector.max(out=EM8[:, k, :], in_=e16[:])
                nc.vector.max_index(out=IDX8[:, k, :], in_max=EM8[:, k, :],
                                    in_values=e16[:])
                # sizes[b, perm] gather; batch base b*1024 folded in as the
                # constant element_offset (b = k//8 is compile-time here)
                nc.gpsimd.indirect_dma_start(
                    out=SZG[:, k:k + 1], out_offset=None, in_=szflat,
                    in_offset=bass.IndirectOffsetOnAxis(ap=IDX8[:, k, 0:1], axis=0),
                    element_offset=(k // (NBLK // BC)) * V,
                )

            # perm + b*1024 for phase 2 (one batched op)
            nc.vector.tensor_tensor(out=SIDX[:], in0=BIOT[:],
                                    in1=IDX8[:, :, 0], op=ALU.add)

            # ---------------- CE partial -----------------------------------
            LSE = sb.tile([P, NBLK], f32)
            nc.scalar.activation(out=LSE[:], in_=SUME[:], func=ACTF.Ln,
                                 bias=0.0, scale=1.0)
            CET = sb.tile([P, NBLK], f32)
            nc.vector.tensor_tensor(out=CET[:], in0=XG[:], in1=LSE[:],
                                    op=ALU.subtract)
            CEcol = sb.tile([P, 1], f32)
            nc.vector.reduce_sum(out=CEcol[:], in_=CET[:], axis=AX.X)

            # ---------------- phase 2: scan + pair count -------------------
            SZF = sb.tile([P, NBLK], f32)
            nc.vector.tensor_copy(out=SZF[:], in_=SZG[:])
            PERMA = sb.tile([P, NBLK], f32)
            nc.vector.tensor_copy(out=PERMA[:], in_=SIDX[:])  # perm + b*1024

            # transposes to [32, 128] (t on free dim within chunk)
            PT1 = ps.tile([NBLK, P], f32, space="PSUM")
            nc.tensor.transpose(out=PT1[:], in_=PERMA[:], identity=ident[:])
            P32 = sb.tile([NBLK, P], f32)
            nc.vector.tensor_copy(out=P32[:], in_=PT1[:])
            PT2 = ps.tile([NBLK, P], f32, space="PSUM")
            nc.tensor.transpose(out=PT2[:], in_=SZF[:], identity=ident[:])
            S32 = sb.tile([NBLK, P], f32)
            nc.vector.tensor_copy(out=S32[:], in_=PT2[:])

            # shifted-by-one-chunk copy (row k <- row k-1; row 0 wraps to row 31
            # whose contribution always cancels via the b*1024 augmentation)
            shmask = [31] + list(range(31))
            SHP = sb.tile([NBLK, P], f32)
            nc.vector.stream_shuffle(out=SHP[:], in_=P32[:], mask=shmask)

            # same-station flags vs previous slot (aug makes cross-batch False)
            SAME = sb.tile([NBLK, P], f32)
            nc.vector.tensor_tensor(out=SAME[:, 1:P], in0=P32[:, 1:P],
                                    in1=P32[:, 0:P - 1], op=ALU.is_equal)
            nc.vector.tensor_tensor(out=SAME[:, 0:1], in0=P32[:, 0:1],
                                    in1=SHP[:, P - 1:P], op=ALU.is_equal)

            # a_t = same ? s_t - 700 : -BIG   (exact integer algebra in f32)
            A32 = sb.tile([NBLK, P], f32)
            nc.vector.tensor_scalar(out=A32[:], in0=S32[:], scalar1=BIG - TAKT,
                                    scalar2=None, op0=ALU.add)
            nc.vector.tensor_tensor(out=A32[:], in0=A32[:], in1=SAME[:],
                                    op=ALU.mult)
            nc.vector.tensor_scalar(out=A32[:], in0=A32[:], scalar1=BIG,
                                    scalar2=None, op0=ALU.subtract)

            # level-1 scan within chunks
            E1 = sb.tile([NBLK, P], f32)
            nc.vector.tensor_tensor_scan(out=E1[:], data0=A32[:], data1=S32[:],
                                         initial=NEG, op0=ALU.add, op1=ALU.max)
            ACOL = sb.tile([NBLK, 1], f32)
            nc.vector.reduce_sum(out=ACOL[:], in_=A32[:], axis=AX.X)
            BCOL = sb.tile([NBLK, 1], f32)
            nc.vector.tensor_copy(out=BCOL[:], in_=E1[:, P - 1:P])

            # level-2 scan across the 32 chunk maps (cols -> rows via matmul)
            PA = ps.tile([1, NBLK], f32, space="PSUM")
            nc.tensor.matmul(out=PA[:], lhsT=ACOL[:],
                             rhs=ident[0:NBLK, 0:NBLK], start=True, stop=True)
            PB = ps.tile([1, NBLK], f32, space="PSUM")
            nc.tensor.matmul(out=PB[:], lhsT=BCOL[:],
                             rhs=ident[0:NBLK, 0:NBLK], start=True, stop=True)
            ASB = sb.tile([1, NBLK], f32)
            nc.vector.tensor_copy(out=ASB[:], in_=PA[:])
            BSB = sb.tile([1, NBLK], f32)
            nc.vector.tensor_copy(out=BSB[:], in_=PB[:])
            S2 = sb.tile([1, NBLK], f32)
            nc.vector.tensor_tensor_scan(out=S2[:], data0=ASB[:],
                                         data1=BSB[:], initial=NEG,
                                         op0=ALU.add, op1=ALU.max)
            EINR = sb.tile([1, NBLK], f32)
            nc.vector.memset(EINR[:, 0:1], NEG)
            nc.vector.tensor_copy(out=EINR[:, 1:NBLK], in_=S2[:, 0:NBLK - 1])
            PEIN = ps.tile([NBLK, 1], f32, space="PSUM")
            nc.tensor.matmul(out=PEIN[:], lhsT=EINR[:], rhs=ones11[:],
                             start=True, stop=True)
            EIN = sb.tile([NBLK, 1], f32)
            nc.vector.tensor_copy(out=EIN[:], in_=PEIN[:])

            # level-3: exact e per slot; xe = 700u + e, xs = xe - s
            E = sb.tile([NBLK, P], f32)
            nc.vector.tensor_tensor_scan(out=E[:], data0=A32[:], data1=S32[:],
                                         initial=EIN[:], op0=ALU.add, op1=ALU.max)
            XE = sb.tile([NBLK, P], f32)
            nc.vector.tensor_tensor(out=XE[:], in0=E[:], in1=U700[:], op=ALU.add)
            XS = sb.tile([NBLK, P], f32)
            nc.vector.tensor_tensor(out=XS[:], in0=XE[:], in1=S32[:],
                                    op=ALU.subtract)
            SHXE = sb.tile([NBLK, P], f32)
            nc.vector.stream_shuffle(out=SHXE[:], in_=XE[:], mask=shmask)
            SHXS = sb.tile([NBLK, P], f32)
            nc.vector.stream_shuffle(out=SHXS[:], in_=XS[:], mask=shmask)

            # pair count for d in [2, W] (wrap compares on Pool, main on DVE)
            NACC = 2 * (W - 1)
            ACC = sb.tile([NBLK, NACC], f32)
            nc.vector.memset(ACC[:], 0.0)
            CJ = sb.tile([NBLK, P], f32)
            C2 = sb.tile([NBLK, P], f32)
            C4 = sb.tile([NBLK, P], f32)
            WJ = sb.tile([NBLK, P], f32)
            W2 = sb.tile([NBLK, P], f32)
            W4 = sb.tile([NBLK, P], f32)
            for d in range(2, W + 1):
                col = 2 * (d - 2)
                n = P - d
                # main: t = (k, p>=d), t' = (k, p-d)
                nc.vector.tensor_tensor(out=CJ[:, :n], in0=P32[:, d:P],
                                        in1=P32[:, 0:n], op=ALU.is_equal)
                nc.vector.tensor_tensor(out=C2[:, :n], in0=XS[:, d:P],
                                        in1=XE[:, 0:n], op=ALU.is_lt)
                nc.vector.tensor_tensor(out=CJ[:, :n], in0=CJ[:, :n],
                                        in1=C2[:, :n], op=ALU.mult)
                nc.vector.tensor_tensor(out=C4[:, :n], in0=XE[:, d:P],
                                        in1=XS[:, 0:n], op=ALU.is_gt)
                nc.vector.tensor_tensor(out=CJ[:, :n], in0=CJ[:, :n],
                                        in1=C4[:, :n], op=ALU.mult)
                nc.vector.reduce_sum(out=ACC[:, col:col + 1], in_=CJ[:, :n],
                                     axis=AX.X)
                # wrap: t = (k, p<d), t' = (k-1, 128-d+p); row 0 self-cancels
                nc.vector.tensor_tensor(out=WJ[:, :d], in0=P32[:, 0:d],
                                        in1=SHP[:, P - d:P], op=ALU.is_equal)
                nc.vector.tensor_tensor(out=W2[:, :d], in0=XS[:, 0:d],
                                        in1=SHXE[:, P - d:P], op=ALU.is_lt)
                nc.vector.tensor_tensor(out=WJ[:, :d], in0=WJ[:, :d],
                                        in1=W2[:, :d], op=ALU.mult)
                nc.vector.tensor_tensor(out=W4[:, :d], in0=XE[:, 0:d],
                                        in1=SHXS[:, P - d:P], op=ALU.is_gt)
                nc.vector.tensor_tensor(out=WJ[:, :d], in0=WJ[:, :d],
                                        in1=W4[:, :d], op=ALU.mult)
                nc.vector.reduce_sum(out=ACC[:, col + 1:col + 2],
                                     in_=WJ[:, :d], axis=AX.X)

            CNT = sb.tile([NBLK, 1], f32)
            nc.vector.reduce_sum(out=CNT[:], in_=ACC[:], axis=AX.X)

            # ---------------- partial sums out -----------------------------
            PSC = ps.tile([1, 2], f32, space="PSUM")
            nc.tensor.matmul(out=PSC[:, 0:1], lhsT=CEcol[:], rhs=ones128[:],
                             start=True, stop=True)
            nc.tensor.matmul(out=PSC[:, 1:2], lhsT=CNT[:],
                             rhs=ones128[0:NBLK, :], start=True, stop=True)
            OUTSB = sb.tile([1, 2], f32)
            nc.vector.tensor_copy(out=OUTSB[:], in_=PSC[:])
            nc.sync.dma_start(out=outd.ap(), in_=OUTSB[:])

    nc.compile()
    return nc


_NC_CACHE = None


def _get_program():
    global _NC_CACHE
    if _NC_CACHE is None:
        _NC_CACHE = _build_program()
    return _NC_CACHE


def kernel(logits: np.ndarray, tgt: np.ndarray, sizes: np.ndarray) -> np.ndarray:
    logits = np.ascontiguousarray(np.asarray(logits, np.float32))
    tgt = np.ascontiguousarray(np.asarray(tgt, np.int32))
    sizes = np.ascontiguousarray(np.asarray(sizes, np.int32))
    assert logits.shape == (B, T, V)

    nc = _get_program()
    in_maps = []
    for i in range(NCORES):
        s = slice(i * BC, (i + 1) * BC)
        in_maps.append({
            "logits": logits[s],
            "tgt": tgt[s],
            "sizes": sizes[s],
        })
    res = bass_utils.run_bass_kernel_spmd(nc, in_maps, core_ids=list(range(NCORES)))
    ce_sum = 0.0
    cnt_sum = 0.0
    for r in res.results:
        o = r["out"]
        ce_sum += float(o[0, 0])
        cnt_sum += float(o[0, 1])
    loss = -(ce_sum) / (B * T) + cnt_sum / B
    return np.asarray(loss, dtype=np.float32)


# revision 10
# speedup vs baseline: 1.0126x; 1.0126x over previous
"""Trainium2 Bass kernel for nn_CustomLoss_38062000177852.

Computes: CE(logits, tgt) + overlap_penalty(argmax(logits), sizes) for
logits [32,1024,1024] f32, tgt [32,1024] i32, sizes [32,1024] i32.

Sharding: batch dim (32) split 4-per-core across 8 NeuronCores (SPMD, one
Bass program, per-core input shards). Each core returns two partial sums
(ce_sum, overlap_count); host combines: loss = -ce/(B*T) + count/B.

Per-core layout: 4096 rows (b,t) -> 32 blocks of 128 rows. Row (b,t) lives
at partition p = t%128 of block k = b*8 + t//128 (flat row k*128+p).

v2 engine schedule (per streamed block, ~1.4 us steady state):
  DMA  : xk [128,1024] f32 HBM->SBUF (ring of 4).
  ACT  : exps16 = Exp(xk) in fp16 with f32 accum -> sum-exp. fp16 exp is
         monotone in x, so all later max/argmax run on exps16 at DVE 2x rate.
  DVE  : max8(exps16) -> top-8, max_index -> argmax column (2x fp16).
  DVE/Pool (split): x[tgt] extraction as a fused mask-dot:
         XGE[p,k] = sum_v [v==tgt] * exps16[p,v]  (= exp(x[tgt]));
         x[tgt] = Ln(XGE) later on ACT. One scalar_tensor_tensor pass.
  Pool : sizes[b, perm] softDGE indirect gather (128 offsets/call).
CE = sum(Ln(XGE) - Ln(sum-exp)); then the offset scan + pair count (below).

Offset recurrence (reference scan): e_t = s_t + same_t*max(e_{t-1}-700, 0)
rewritten as e_t = max(e_{t-1} + a_t, b_t), a_t = same_t ? s_t-700 : -BIG,
b_t = s_t  -- a (max,+) linear scan, computed hierarchically: per-chunk scan
([32,128], t on free dim), chunk-map composition scan over 32 chunks, then
re-scan with per-chunk initial states. Exact in f32 (all values < 2^24).

Overlap count: pairs (t, t-d) need 700d < e_{t-d} - offs_t, so only d <= W
can overlap (adjacent d=1 provably never overlaps). Counted with shifted-AP
compares for d in [2, W].
"""
import numpy as np

import concourse.bacc as bacc
import concourse.bass as bass
import concourse.mybir as mybir
import concourse.tile as tile
from concourse import bass_utils
from concourse.masks import make_identity

f32 = mybir.dt.float32
f16 = mybir.dt.float16
i32 = mybir.dt.int32
u32 = mybir.dt.uint32
ALU = mybir.AluOpType
AX = mybir.AxisListType
ACTF = mybir.ActivationFunctionType

B, T, V = 32, 1024, 1024
NCORES = 8
BC = B // NCORES              # batches per core
NBLK = BC * (T // 128)        # 32 row-blocks per core
P = 128
TAKT = 700.0
BIG = 1.0e6                   # absorbing "minus infinity" for the scan input
NEG = -1.0e30                 # scan initial state
W = 6                         # max pair distance checked (d in [2, W])
XG_POOL = 0                   # blocks whose x[tgt] mask-dot runs on Pool
                              # (codegen rejects TensorScalarPtr on Pool)


def _build_program():
    nc = bacc.Bacc("TRN2", debug=False)

    lg = nc.dram_tensor("logits", [BC, T, V], f32, kind="ExternalInput")
    tg = nc.dram_tensor("tgt", [BC, T], i32, kind="ExternalInput")
    sz = nc.dram_tensor("sizes", [BC, V], i32, kind="ExternalInput")
    outd = nc.dram_tensor("out", [1, 2], f32, kind="ExternalOutput")

    lgf = lg.ap().rearrange("b t v -> (b t) v")          # [4096, 1024]
    lgflat = lg.ap().rearrange("b t v -> (b t v)").rearrange("(n o) -> n o", o=1)
    szflat = sz.ap().rearrange("b v -> (b v)").rearrange("(n o) -> n o", o=1)

    with tile.TileContext(nc) as tc:
        with (
            tc.tile_pool(name="sb", bufs=1) as sb,
            tc.tile_pool(name="xring", bufs=6) as xring,
            tc.tile_pool(name="ering", bufs=4) as ering,
            tc.tile_pool(name="ps", bufs=1, space="PSUM") as ps,
        ):
            # ---------------- constants / early independent work ----------
            ident = sb.tile([P, P], f32)
            make_identity(nc, ident)
            ones128 = sb.tile([P, 1], f32)
            nc.vector.memset(ones128[:], 1.0)
            ones11 = sb.tile([1, 1], f32)
            nc.vector.memset(ones11[:], 1.0)

            # tgt in [p, (b,c)] layout via strided DMA
            TGT = sb.tile([P, NBLK], i32)
            nc.sync.dma_start(
                out=TGT[:].rearrange("p (b c) -> p b c", b=BC),
                in_=tg.ap().rearrange("b (c p) -> p b c", p=P),
            )
            # x[tgt] gather offsets: flat = (k*128+p)*1024 + tgt
            OFB = sb.tile([P, NBLK], i32)
            nc.gpsimd.iota(OFB[:], pattern=[[P, NBLK]], base=0,
                           channel_multiplier=1)
            nc.vector.tensor_scalar(out=OFB[:], in0=OFB[:], scalar1=float(V),
                                    scalar2=None, op0=ALU.mult)
            OFFX = sb.tile([P, NBLK], i32)
            nc.vector.tensor_tensor(out=OFFX[:], in0=OFB[:], in1=TGT[:], op=ALU.add)
            XG = sb.tile([P, NBLK], f32)
            # front-load all 32 x[tgt] gathers on Pool (independent of compute,
            # so they fill Pool's idle head while SZG gathers trickle in later)
            for k in range(NBLK):
                nc.gpsimd.indirect_dma_start(
                    out=XG[:, k:k + 1], out_offset=None, in_=lgflat,
                    in_offset=bass.IndirectOffsetOnAxis(ap=OFFX[:, k:k + 1], axis=0),
                )

            # b*1024 iota (batch id base for sizes gather / perm augmentation)
            BIOT = sb.tile([P, NBLK], i32)
            nc.gpsimd.iota(BIOT[:].rearrange("p (b c) -> p b c", b=BC),
                           pattern=[[T, BC], [0, NBLK // BC]], base=0,
                           channel_multiplier=0)

            # u*700 grid in [32, 128] layout (u = k*128 + f)
            UI = sb.tile([NBLK, P], i32)
            nc.gpsimd.iota(UI[:], pattern=[[1, P]], base=0, channel_multiplier=P)
            U700 = sb.tile([NBLK, P], f32)
            nc.vector.tensor_scalar(out=U700[:], in0=UI[:], scalar1=TAKT,
                                    scalar2=None, op0=ALU.mult)

            # ---------------- phase 1: stream logits -----------------------
            EM8 = sb.tile([P, NBLK, 8], f16)
            IDX8 = sb.tile([P, NBLK, 8], u32)
            SUME = sb.tile([P, NBLK], f32)
            SIDX = sb.tile([P, NBLK], i32)
            SZG = sb.tile([P, NBLK], i32)

            for k in range(NBLK):
                xk = xring.tile([P, V], f32, tag="x")
                nc.sync.dma_start(out=xk[:], in_=lgf[k * P:(k + 1) * P, :])
                e16 = ering.tile([P, V], f16, tag="e")
                nc.scalar.activation(out=e16[:], in_=xk[:], func=ACTF.Exp,
                                     bias=0.0, scale=1.0,
                                     accum_out=SUME[:, k:k + 1])
                nc.vector.max(out=EM8[:, k, :], in_=e16[:])
                nc.vector.max_index(out=IDX8[:, k, :], in_max=EM8[:, k, :],
                                    in_values=e16[:])
                # sizes[b, perm] gather; batch base b*1024 folded in as the
                # constant element_offset (b = k//8 is compile-time here)
                nc.gpsimd.indirect_dma_start(
                    out=SZG[:, k:k + 1], out_offset=None, in_=szflat,
                    in_offset=bass.IndirectOffsetOnAxis(ap=IDX8[:, k, 0:1], axis=0),
                    element_offset=(k // (NBLK // BC)) * V,
                )

            # perm + b*1024 for phase 2 (one batched op)
            nc.vector.tensor_tensor(out=SIDX[:], in0=BIOT[:],
                                    in1=IDX8[:, :, 0], op=ALU.add)

            # ---------------- CE partial -----------------------------------
            LSE = sb.tile([P, NBLK], f32)
            nc.scalar.activation(out=LSE[:], in_=SUME[:], func=ACTF.Ln,
                                 bias=0.0, scale=1.0)
            CET = sb.tile([P, NBLK], f32)
            nc.vector.tensor_tensor(out=CET[:], in0=XG[:], in1=LSE[:],
                                    op=ALU.subtract)
            CEcol = sb.tile([P, 1], f32)
            nc.vector.reduce_sum(out=CEcol[:], in_=CET[:], axis=AX.X)

            # ---------------- phase 2: scan + pair count -------------------
            SZF = sb.tile([P, NBLK], f32)
            nc.vector.tensor_copy(out=SZF[:], in_=SZG[:])
            PERMA = sb.tile([P, NBLK], f32)
            nc.vector.tensor_copy(out=PERMA[:], in_=SIDX[:])  # perm + b*1024

            # transposes to [32, 128] (t on free dim within chunk), written
            # into col [6:134] of extended tiles whose cols [0:6] hold the
            # previous chunk's last 6 slots (so wrap pairs fold into the main
            # windowed compares below).
            EXT = 6
            PE_ = P + EXT
            EXTP = sb.tile([NBLK, PE_], f32)
            EXTXS = sb.tile([NBLK, PE_], f32)
            EXTXE = sb.tile([NBLK, PE_], f32)
            shmask = [31] + list(range(31))

            PT1 = ps.tile([NBLK, P], f32, space="PSUM")
            nc.tensor.transpose(out=PT1[:], in_=PERMA[:], identity=ident[:])
            P32 = EXTP[:, EXT:PE_]
            nc.vector.tensor_copy(out=P32, in_=PT1[:])
            PT2 = ps.tile([NBLK, P], f32, space="PSUM")
            nc.tensor.transpose(out=PT2[:], in_=SZF[:], identity=ident[:])
            S32 = sb.tile([NBLK, P], f32)
            nc.vector.tensor_copy(out=S32[:], in_=PT2[:])

            # previous-chunk prefix (row k <- row k-1 of last 6 cols; row 0
            # wraps to row 31 whose contribution cancels via the b*1024 aug)
            nc.vector.stream_shuffle(out=EXTP[:, 0:EXT],
                                     in_=EXTP[:, PE_ - EXT:PE_], mask=shmask)

            # same-station flags vs previous slot (aug makes cross-batch False)
            SAME = sb.tile([NBLK, P], f32)
            nc.vector.tensor_tensor(out=SAME[:], in0=EXTP[:, EXT:PE_],
                                    in1=EXTP[:, EXT - 1:PE_ - 1], op=ALU.is_equal)

            # a_t = same ? s_t - 700 : -BIG  ==  (s_t + (BIG-700))*same - BIG
            A32 = sb.tile([NBLK, P], f32)
            nc.vector.scalar_tensor_tensor(out=A32[:], in0=S32[:],
                                           scalar=BIG - TAKT, in1=SAME[:],
                                           op0=ALU.add, op1=ALU.mult)
            nc.vector.tensor_scalar(out=A32[:], in0=A32[:], scalar1=BIG,
                                    scalar2=None, op0=ALU.subtract)

            # level-1 scan within chunks
            E1 = sb.tile([NBLK, P], f32)
            nc.vector.tensor_tensor_scan(out=E1[:], data0=A32[:], data1=S32[:],
                                         initial=NEG, op0=ALU.add, op1=ALU.max)
            ACOL = sb.tile([NBLK, 1], f32)
            nc.vector.reduce_sum(out=ACOL[:], in_=A32[:], axis=AX.X)
            BCOL = sb.tile([NBLK, 1], f32)
            nc.vector.tensor_copy(out=BCOL[:], in_=E1[:, P - 1:P])

            # level-2 scan across the 32 chunk maps (cols -> rows via matmul)
            PA = ps.tile([1, NBLK], f32, space="PSUM")
            nc.tensor.matmul(out=PA[:], lhsT=ACOL[:],
                             rhs=ident[0:NBLK, 0:NBLK], start=True, stop=True)
            PB = ps.tile([1, NBLK], f32, space="PSUM")
            nc.tensor.matmul(out=PB[:], lhsT=BCOL[:],
                             rhs=ident[0:NBLK, 0:NBLK], start=True, stop=True)
            ASB = sb.tile([1, NBLK], f32)
            nc.vector.tensor_copy(out=ASB[:], in_=PA[:])
            BSB = sb.tile([1, NBLK], f32)
            nc.vector.tensor_copy(out=BSB[:], in_=PB[:])
            S2 = sb.tile([1, NBLK], f32)
            nc.vector.tensor_tensor_scan(out=S2[:], data0=ASB[:],
                                         data1=BSB[:], initial=NEG,
                                         op0=ALU.add, op1=ALU.max)
            EINR = sb.tile([1, NBLK], f32)
            nc.vector.memset(EINR[:, 0:1], NEG)
            nc.vector.tensor_copy(out=EINR[:, 1:NBLK], in_=S2[:, 0:NBLK - 1])
            PEIN = ps.tile([NBLK, 1], f32, space="PSUM")
            nc.tensor.matmul(out=PEIN[:], lhsT=EINR[:], rhs=ones11[:],
                             start=True, stop=True)
            EIN = sb.tile([NBLK, 1], f32)
            nc.vector.tensor_copy(out=EIN[:], in_=PEIN[:])

            # level-3: exact e per slot; xe = 700u + e, xs = xe - s
            E = sb.tile([NBLK, P], f32)
            nc.vector.tensor_tensor_scan(out=E[:], data0=A32[:], data1=S32[:],
                                         initial=EIN[:], op0=ALU.add, op1=ALU.max)
            XE = EXTXE[:, EXT:PE_]
            nc.vector.tensor_tensor(out=XE, in0=E[:], in1=U700[:], op=ALU.add)
            XS = EXTXS[:, EXT:PE_]
            nc.vector.tensor_tensor(out=XS, in0=EXTXE[:, EXT:PE_], in1=S32[:],
                                    op=ALU.subtract)
            nc.vector.stream_shuffle(out=EXTXE[:, 0:EXT],
                                     in_=EXTXE[:, PE_ - EXT:PE_], mask=shmask)
            nc.vector.stream_shuffle(out=EXTXS[:, 0:EXT],
                                     in_=EXTXS[:, PE_ - EXT:PE_], mask=shmask)

            # pair count for d in [2, W]: window compares over the extended
            # tiles cover main and wrap pairs in one shot
            NACC = W - 1
            ACC = sb.tile([NBLK, NACC], f32)
            CJ = sb.tile([NBLK, P], f32)
            C2 = sb.tile([NBLK, P], f32)
            C4 = sb.tile([NBLK, P], f32)
            for d in range(2, W + 1):
                col = d - 2
                nc.vector.tensor_tensor(out=CJ[:], in0=EXTP[:, EXT:PE_],
                                        in1=EXTP[:, EXT - d:PE_ - d],
                                        op=ALU.is_equal)
                nc.vector.tensor_tensor(out=C2[:], in0=EXTXS[:, EXT:PE_],
                                        in1=EXTXE[:, EXT - d:PE_ - d],
                                        op=ALU.is_lt)
                nc.vector.tensor_tensor(out=CJ[:], in0=CJ[:], in1=C2[:],
                                        op=ALU.mult)
                nc.vector.tensor_tensor(out=C4[:], in0=EXTXE[:, EXT:PE_],
                                        in1=EXTXS[:, EXT - d:PE_ - d],
                                        op=ALU.is_gt)
                nc.vector.tensor_tensor(out=CJ[:], in0=CJ[:], in1=C4[:],
                                        op=ALU.mult)
                nc.vector.reduce_sum(out=ACC[:, col:col + 1], in_=CJ[:],
                                     axis=AX.X)

            CNT = sb.tile([NBLK, 1], f32)
            nc.vector.reduce_sum(out=CNT[:], in_=ACC[:], axis=AX.X)

            # ---------------- partial sums out -----------------------------
            PSC = ps.tile([1, 2], f32, space="PSUM")
            nc.tensor.matmul(out=PSC[:, 0:1], lhsT=CEcol[:], rhs=ones128[:],
                             start=True, stop=True)
            nc.tensor.matmul(out=PSC[:, 1:2], lhsT=CNT[:],
                             rhs=ones128[0:NBLK, :], start=True, stop=True)
            OUTSB = sb.tile([1, 2], f32)
            nc.vector.tensor_copy(out=OUTSB[:], in_=PSC[:])
            nc.sync.dma_start(out=outd.ap(), in_=OUTSB[:])

    nc.compile()
    return nc


_NC_CACHE = None


def _get_program():
    global _NC_CACHE
    if _NC_CACHE is None:
        _NC_CACHE = _build_program()
    return _NC_CACHE


def kernel(logits: np.ndarray, tgt: np.ndarray, sizes: np.ndarray) -> np.ndarray:
    logits = np.ascontiguousarray(np.asarray(logits, np.float32))
    tgt = np.ascontiguousarray(np.asarray(tgt, np.int32))
    sizes = np.ascontiguousarray(np.asarray(sizes, np.int32))
    assert logits.shape == (B, T, V)

    nc = _get_program()
    in_maps = []
    for i in range(NCORES):
        s = slice(i * BC, (i + 1) * BC)
        in_maps.append({
            "logits": logits[s],
            "tgt": tgt[s],
            "sizes": sizes[s],
        })
    res = bass_utils.run_bass_kernel_spmd(nc, in_maps, core_ids=list(range(NCORES)))
    ce_sum = 0.0
    cnt_sum = 0.0
    for r in res.results:
        o = r["out"]
        ce_sum += float(o[0, 0])
        cnt_sum += float(o[0, 1])
    loss = -(ce_sum) / (B * T) + cnt_sum / B
    return np.asarray(loss, dtype=np.float32)
